# revision 7
# baseline (speedup 1.0000x reference)
"""BitLinear kernel for Trainium2, 8-core column-parallel.

Computes out = x @ (sign(W) * (weight_scale @ input_factor)).T
  x: [32, 8, 4096] f32, W: [11008, 4096] f32,
  weight_scale: [11008, 4] f32, input_factor: [4, 4096] f32
  -> out: [32, 8, 11008] f32

Sharding: column-parallel over out_features (11008 = 8 x 1376). Each core
gets its W / weight_scale row-shard plus replicated x / input_factor, and
produces out[:, core_slice]; host concatenates. No collectives.

Per-core dataflow (all on-device):
  - W streamed from DRAM in [128, 2048] tiles (natural [o, i] layout)
  - PE transposes each [128, 128] sub-tile -> PSUM (i on partitions)
  - PE computes value strips value[i_blk, o_chunk] = f.T @ wsT  (K=4 matmul)
  - ACT evacuates transposed W PSUM -> SBUF
  - DVE applies the sign in one fused op: w_signed = (wT & -0.0) ^ value
  - PE main matmul: out[t, o] += xT_blk.T @ w_signed, accumulated in PSUM
    over all 32 i-blocks, then evacuated + DMA'd out.
"""

import os
import sys

if "/opt/trn_rl_repo" not in sys.path:
    sys.path.insert(0, "/opt/trn_rl_repo")

import numpy as np

# ---------------------------------------------------------------------------
# problem constants (hardcoded per the self-contained-kernel contract)
B, S, IN, OUT, R = 32, 8, 4096, 11008, 4
T = B * S               # 256 tokens
NCORES = 8
OS = OUT // NCORES      # 1376 out-features per core
P = 128
IC = 2048               # i-span per W DMA macro-tile
O_CHUNKS = [(0, 512), (512, 512), (1024, 352)]


def _install_tile_drain_patch():
    """This walrus build rejects >2 sync waits on one TPB_CTRL instruction;
    split the TileContext end-of-kernel drain into one drain per proc."""
    from concourse.tile import TileContext
    from concourse.vector_clock import ScopedClock
    from bass_rust import VectorClock

    if getattr(TileContext, "_drain_patch_installed", False):
        return

    def patched_drain_and_barrier(self, tick_clock, wait_clock):
        nc = self.nc
        gc = tick_clock.global_clock
        for i in range(27):
            v = gc[i]
            if v > 0:
                single = [0] * 27
                single[i] = v
                d = nc.sync.drain()
                wait_clock.add_sem_waits(
                    d.ins, ScopedClock({None: VectorClock(single)})
                )
        nc.all_engine_barrier()
        assert self.sems is not None
        popped = nc._tile_sem_poison_stack.pop()
        assert popped is self._sem_poison
        nc.clear_and_free_semaphores(list(self.sems.allocated().values()))
        nc.all_engine_barrier()

    TileContext._drain_and_barrier = patched_drain_and_barrier
    TileContext._drain_patch_installed = True


def _split_excess_waits(nc, max_waits=1):
    """This walrus build rejects instructions carrying more than ~2 sync
    waits. Move excess waits onto no-op instructions inserted immediately
    before the offender on the same engine (same semantics: the engine
    performs the same waits, in order, before executing the instruction)."""
    import concourse.mybir as mybir

    n_split = 0
    for fn in nc.m.functions:
        for bb in fn.blocks:
            insts = list(bb.instructions)
            new = []
            changed = False
            for inst in insts:
                si = inst.sync_info
                waits = list(si.on_wait) if si is not None else []
                if len(waits) > max_waits:
                    changed = True
                    n_split += 1
                    excess = waits[:-max_waits]
                    keep = waits[-max_waits:]
                    for i in range(0, len(excess), max_waits):
                        chunk = excess[i : i + max_waits]
                        nop = mybir.InstNoOp(
                            name=nc.get_next_instruction_name(),
                            sync_info=mybir.SyncInfo(
                                on_wait=chunk, on_update=[]
                            ),
                            bass_nofuse=True,
                            engine=inst.engine,
                        )
                        new.append(nop)
                    inst.sync_info = mybir.SyncInfo(
                        on_wait=keep, on_update=list(si.on_update)
                    )
                new.append(inst)
            if changed:
                bb.instructions = new
    return n_split


def build_nc():
    import concourse.bass as bass
    import concourse.mybir as mybir
    from concourse.bass import ts
    from concourse.masks import make_identity
    from concourse.tile import TileContext

    _install_tile_drain_patch()

    DT = mybir.dt.float32
    nc = bass.Bass("TRN2", num_devices=NCORES)

    w_ext = nc.dram_tensor("w", [OS, IN], DT, kind="ExternalInput").ap()
    xT_ext = nc.dram_tensor("xT", [IN, T], DT, kind="ExternalInput").ap()
    wsT_ext = nc.dram_tensor("wsT", [R, OS], DT, kind="ExternalInput").ap()
    f_ext = nc.dram_tensor("f", [R, IN], DT, kind="ExternalInput").ap()
    out_ext = nc.dram_tensor("out", [T, OS], DT, kind="ExternalOutput").ap()

    with TileContext(nc) as tc:
        with (
            tc.tile_pool(name="const", bufs=1) as cpool,
            tc.tile_pool(name="wpool", bufs=2) as wpool,
            tc.tile_pool(name="wtpool", bufs=3) as wtpool,
            tc.tile_pool(name="wsgpool", bufs=3) as wsgpool,
            tc.tile_pool(name="outsb", bufs=2) as outsb,
            tc.tile_pool(name="tpsum", bufs=2, space="PSUM") as tpool,
            tc.tile_pool(name="vpsum", bufs=2, space="PSUM") as vpool,
            tc.tile_pool(name="opsum", bufs=4, space="PSUM") as opool,
        ):
            identity = cpool.tile([P, P], DT)
            make_identity(nc, identity)

            f_sb = cpool.tile([R, IN], DT)
            nc.gpsimd.dma_start(f_sb[:, :], f_ext[:, :])
            wsT_sb = cpool.tile([R, OS], DT)
            nc.gpsimd.dma_start(wsT_sb[:, :], wsT_ext[:, :])

            # resident xT: [128, 32, 256], block ib holds xT[ib*128:(ib+1)*128, :]
            xT_sb = cpool.tile([P, IN // P, T], DT)
            nc.gpsimd.dma_start(
                xT_sb[:, :, :], xT_ext.rearrange("(a p) t -> p a t", p=P)
            )

            n_iblk = IN // P  # 32
            for (o0, No) in O_CHUNKS:
                nsub = (No + P - 1) // P
                out_ps = [
                    opool.tile([P, No], DT, tag="out_ps", name=f"out_ps{tb}")
                    for tb in range(2)
                ]
                for i0 in range(0, IN, IC):
                    w_sb = wpool.tile([P, nsub, IC], DT, tag="w_sb")
                    for j in range(nsub):
                        rows = min(P, No - j * P)
                        nc.sync.dma_start(
                            w_sb[:rows, j],
                            w_ext[o0 + j * P : o0 + j * P + rows, i0 : i0 + IC],
                        )
                    for ib4 in range(IC // P):
                        ib = i0 // P + ib4
                        first = ib == 0
                        last = ib == n_iblk - 1
                        wT_ps = tpool.tile([P, No], DT)
                        for j in range(nsub):
                            rows = min(P, No - j * P)
                            nc.tensor.transpose(
                                wT_ps[:, j * P : j * P + rows],
                                w_sb[:rows, j, ts(ib4, P)],
                                identity[:rows, :rows],
                            )
                        value_ps = vpool.tile([P, No], DT)
                        nc.tensor.matmul(
                            value_ps,
                            f_sb[:, ts(ib, P)],
                            wsT_sb[:, o0 : o0 + No],
                            start=True,
                            stop=True,
                        )
                        # evacuate the transposed W strip through ACT's Sign
                        # LUT: s = sign(wT) in {-1, 0, +1} (sign(+-0) = 0,
                        # matching jnp.sign), then one DVE multiply with the
                        # value strip: w_signed = s * value (exact fp32).
                        s_sb = wtpool.tile([P, No], DT, tag="s_sb")
                        nc.scalar.activation(
                            s_sb, wT_ps, mybir.ActivationFunctionType.Sign
                        )
                        wsg_sb = wsgpool.tile([P, No], DT, tag="wsg_sb")
                        nc.vector.tensor_mul(wsg_sb, s_sb, value_ps)
                        for tb in range(2):
                            nc.tensor.matmul(
                                out_ps[tb],
                                xT_sb[:, ib, ts(tb, P)],
                                wsg_sb,
                                start=first,
                                stop=last,
                            )
                for tb in range(2):
                    o_sb = outsb.tile([P, No], DT, tag="o_sb")
                    nc.scalar.copy(o_sb, out_ps[tb])
                    nc.sync.dma_start(
                        out_ext[ts(tb, P), o0 : o0 + No], o_sb
                    )

    _split_excess_waits(nc)
    return nc


_NC_CACHE = None


def kernel(x, weight, weight_scale, input_factor):
    global _NC_CACHE
    from concourse.bass_utils import run_bass_kernel_spmd

    if _NC_CACHE is None:
        _NC_CACHE = build_nc()
    nc = _NC_CACHE

    xT = np.ascontiguousarray(x.reshape(T, IN).T.astype(np.float32))
    f = np.ascontiguousarray(input_factor.astype(np.float32))
    in_maps = []
    for c in range(NCORES):
        sl = slice(c * OS, (c + 1) * OS)
        in_maps.append(
            {
                "w": np.ascontiguousarray(weight[sl].astype(np.float32)),
                "xT": xT,
                "wsT": np.ascontiguousarray(weight_scale[sl].T.astype(np.float32)),
                "f": f,
            }
        )

    res = run_bass_kernel_spmd(nc, in_maps, core_ids=list(range(NCORES)))
    outs = [res.results[c]["out"] for c in range(NCORES)]
    full = np.concatenate(outs, axis=1)  # [T, OUT]
    return np.ascontiguousarray(full.reshape(B, S, OUT).astype(np.float32))


if __name__ == "__main__":
    # quick self-run with random data
    rng = np.random.default_rng(0)
    x = rng.standard_normal((B, S, IN), dtype=np.float32)
    w = rng.standard_normal((OUT, IN), dtype=np.float32)
    ws = rng.standard_normal((OUT, R), dtype=np.float32)
    f = rng.standard_normal((R, IN), dtype=np.float32)
    out = kernel(x=x, weight=w, weight_scale=ws, input_factor=f)
    wv = ws @ f
    expected = np.einsum("bsi,oi->bso", x, np.sign(w) * wv)
    rel = np.abs(out - expected).max() / np.abs(expected).max()
    print("rel err:", rel)


# revision 14
# speedup vs baseline: 1.2033x; 1.2033x over previous
"""BitLinear kernel for Trainium2, 8-core column-parallel.

Computes out = x @ (sign(W) * (weight_scale @ input_factor)).T
  x: [32, 8, 4096] f32, W: [11008, 4096] f32,
  weight_scale: [11008, 4] f32, input_factor: [4, 4096] f32
  -> out: [32, 8, 11008] f32

Sharding: column-parallel over out_features (11008 = 8 x 1376). Each core
gets its W / weight_scale row-shard plus replicated x / input_factor, and
produces out[:, core_slice]; host concatenates. No collectives.

Per-core dataflow (all on-device):
  - W streamed from DRAM in [128, 2048] tiles (natural [o, i] layout)
  - PE transposes each [128, 128] sub-tile -> PSUM (i on partitions)
  - PE computes value strips value[i_blk, o_chunk] = f.T @ wsT  (K=4 matmul)
  - ACT evacuates transposed W PSUM -> SBUF
  - DVE applies the sign in one fused op: w_signed = (wT & -0.0) ^ value
  - PE main matmul: out[t, o] += xT_blk.T @ w_signed, accumulated in PSUM
    over all 32 i-blocks, then evacuated + DMA'd out.
"""

import os
import sys

if "/opt/trn_rl_repo" not in sys.path:
    sys.path.insert(0, "/opt/trn_rl_repo")

import numpy as np

# ---------------------------------------------------------------------------
# problem constants (hardcoded per the self-contained-kernel contract)
B, S, IN, OUT, R = 32, 8, 4096, 11008, 4
T = B * S               # 256 tokens
NCORES = 8
OS = OUT // NCORES      # 1376 out-features per core
P = 128
IC = 2048               # i-span per W DMA macro-tile
O_CHUNKS = [(0, 512), (512, 512), (1024, 352)]

# matmul precision mode:
#   "f32"  - plain fp32 matmuls (exact, ~1e-6 rel err) but TensorE runs
#            fp32 at 4 cycles/row -> PE-bound ~340us.
#   "f32r" - TF32-like fp32r (11 mantissa bits, 1 cycle/row at N>=256),
#            ~5e-4 rel err, ~3x faster. Well inside the 2e-2 gate.
PRECISION = os.environ.get("BITLINEAR_PRECISION", "f32r")


def _install_tile_drain_patch():
    """This walrus build rejects >2 sync waits on one TPB_CTRL instruction;
    split the TileContext end-of-kernel drain into one drain per proc."""
    from concourse.tile import TileContext
    from concourse.vector_clock import ScopedClock
    from bass_rust import VectorClock

    if getattr(TileContext, "_drain_patch_installed", False):
        return

    def patched_drain_and_barrier(self, tick_clock, wait_clock):
        nc = self.nc
        gc = tick_clock.global_clock
        for i in range(27):
            v = gc[i]
            if v > 0:
                single = [0] * 27
                single[i] = v
                d = nc.sync.drain()
                wait_clock.add_sem_waits(
                    d.ins, ScopedClock({None: VectorClock(single)})
                )
        nc.all_engine_barrier()
        assert self.sems is not None
        popped = nc._tile_sem_poison_stack.pop()
        assert popped is self._sem_poison
        nc.clear_and_free_semaphores(list(self.sems.allocated().values()))
        nc.all_engine_barrier()

    TileContext._drain_and_barrier = patched_drain_and_barrier
    TileContext._drain_patch_installed = True


def _split_excess_waits(nc, max_waits=1):
    """This walrus build rejects instructions carrying more than ~2 sync
    waits. Move excess waits onto no-op instructions inserted immediately
    before the offender on the same engine (same semantics: the engine
    performs the same waits, in order, before executing the instruction)."""
    import concourse.mybir as mybir

    n_split = 0
    for fn in nc.m.functions:
        for bb in fn.blocks:
            insts = list(bb.instructions)
            new = []
            changed = False
            for inst in insts:
                si = inst.sync_info
                waits = list(si.on_wait) if si is not None else []
                if len(waits) > max_waits:
                    changed = True
                    n_split += 1
                    excess = waits[:-max_waits]
                    keep = waits[-max_waits:]
                    for i in range(0, len(excess), max_waits):
                        chunk = excess[i : i + max_waits]
                        nop = mybir.InstNoOp(
                            name=nc.get_next_instruction_name(),
                            sync_info=mybir.SyncInfo(
                                on_wait=chunk, on_update=[]
                            ),
                            bass_nofuse=True,
                            engine=inst.engine,
                        )
                        new.append(nop)
                    inst.sync_info = mybir.SyncInfo(
                        on_wait=keep, on_update=list(si.on_update)
                    )
                new.append(inst)
            if changed:
                bb.instructions = new
    return n_split


def build_nc():
    import concourse.bass as bass
    import concourse.mybir as mybir
    from concourse.bass import ts
    from concourse.masks import make_identity
    from concourse.tile import TileContext

    _install_tile_drain_patch()

    DT = mybir.dt.float32
    # fp32r requires every matmul operand to come from an instruction whose
    # output dtype is float32r (that cast IS the rounding).
    MDT = mybir.dt.float32r if PRECISION == "f32r" else DT
    nc = bass.Bass("TRN2", num_devices=NCORES)

    # xT/wsT/f are pre-rounded to the fp32r grid on the host in f32r mode,
    # and their DRAM tensors declared float32r so the DMA is a valid fp32r
    # producer for the matmuls.
    w_ext = nc.dram_tensor("w", [OS, IN], DT, kind="ExternalInput").ap()
    xT_ext = nc.dram_tensor("xT", [IN, T], MDT, kind="ExternalInput").ap()
    wsT_ext = nc.dram_tensor("wsT", [R, OS], MDT, kind="ExternalInput").ap()
    f_ext = nc.dram_tensor("f", [R, IN], MDT, kind="ExternalInput").ap()
    out_ext = nc.dram_tensor("out", [T, OS], DT, kind="ExternalOutput").ap()

    with TileContext(nc) as tc:
        with (
            tc.tile_pool(name="const", bufs=1) as cpool,
            tc.tile_pool(name="wpool", bufs=3) as wpool,
            tc.tile_pool(name="wtpool", bufs=3) as wtpool,
            tc.tile_pool(name="wsgpool", bufs=3) as wsgpool,
            tc.tile_pool(name="outsb", bufs=2) as outsb,
            tc.tile_pool(name="tpsum", bufs=2, space="PSUM") as tpool,
            tc.tile_pool(name="vpsum", bufs=2, space="PSUM") as vpool,
            tc.tile_pool(name="opsum", bufs=4, space="PSUM") as opool,
        ):
            identity = cpool.tile([P, P], DT)
            make_identity(nc, identity)

            # preloads ride the scalar-engine HWDGE ring; the W stream owns
            # the sync-engine ring.
            f_sb = cpool.tile([R, IN], MDT)
            nc.scalar.dma_start(f_sb[:, :], f_ext[:, :])
            wsT_sb = cpool.tile([R, OS], MDT)
            nc.scalar.dma_start(wsT_sb[:, :], wsT_ext[:, :])

            # resident xT: [128, 32, 256], block ib holds xT[ib*128:(ib+1)*128, :]
            xT_sb = cpool.tile([P, IN // P, T], MDT)
            nc.scalar.dma_start(
                xT_sb[:, :, :], xT_ext.rearrange("(a p) t -> p a t", p=P)
            )

            n_iblk = IN // P  # 32
            for (o0, No) in O_CHUNKS:
                nsub = (No + P - 1) // P
                out_ps = [
                    opool.tile([P, No], DT, tag="out_ps", name=f"out_ps{tb}")
                    for tb in range(2)
                ]
                for i0 in range(0, IN, IC):
                    w_sb = wpool.tile([P, nsub, IC], DT, tag="w_sb")
                    for j in range(nsub):
                        rows = min(P, No - j * P)
                        nc.sync.dma_start(
                            w_sb[:rows, j],
                            w_ext[o0 + j * P : o0 + j * P + rows, i0 : i0 + IC],
                        )
                    for ib4 in range(IC // P):
                        ib = i0 // P + ib4
                        first = ib == 0
                        last = ib == n_iblk - 1
                        wT_ps = tpool.tile([P, No], DT)
                        for j in range(nsub):
                            rows = min(P, No - j * P)
                            nc.tensor.transpose(
                                wT_ps[:, j * P : j * P + rows],
                                w_sb[:rows, j, ts(ib4, P)],
                                identity[:rows, :rows],
                            )
                        value_ps = vpool.tile([P, No], DT)
                        nc.tensor.matmul(
                            value_ps,
                            f_sb[:, ts(ib, P)],
                            wsT_sb[:, o0 : o0 + No],
                            start=True,
                            stop=True,
                        )
                        # evacuate the transposed W strip through ACT's Sign
                        # LUT: s = sign(wT) in {-1, 0, +1} (sign(+-0) = 0,
                        # matching jnp.sign), then one DVE multiply with the
                        # value strip: w_signed = s * value (exact fp32).
                        s_sb = wtpool.tile([P, No], DT, tag="s_sb")
                        nc.scalar.activation(
                            s_sb, wT_ps, mybir.ActivationFunctionType.Sign
                        )
                        # the DVE output cast doubles as the fp32r rounding
                        wsg_sb = wsgpool.tile([P, No], MDT, tag="wsg_sb")
                        nc.vector.tensor_mul(wsg_sb, s_sb, value_ps)
                        for tb in range(2):
                            nc.tensor.matmul(
                                out_ps[tb],
                                xT_sb[:, ib, ts(tb, P)],
                                wsg_sb,
                                start=first,
                                stop=last,
                            )
                for tb in range(2):
                    o_sb = outsb.tile([P, No], DT, tag="o_sb")
                    nc.scalar.copy(o_sb, out_ps[tb])
                    nc.scalar.dma_start(
                        out_ext[ts(tb, P), o0 : o0 + No], o_sb
                    )

    _split_excess_waits(nc)
    return nc


_NC_CACHE = None


def round_f32r(a):
    """Round fp32 to the fp32r grid (11 explicit mantissa bits, RNE) --
    what the on-device fp32r cast would produce."""
    if PRECISION != "f32r":
        return a
    bits = np.ascontiguousarray(a, dtype=np.float32).view(np.uint32)
    drop = 12
    q = np.uint32(1 << drop)
    lsb = (bits >> drop) & 1
    rounded = (bits + (q >> 1) - 1 + lsb) & ~(q - np.uint32(1))
    return rounded.view(np.float32)


def make_in_maps(x, weight, weight_scale, input_factor):
    xT = round_f32r(
        np.ascontiguousarray(x.reshape(T, IN).T.astype(np.float32))
    )
    f = round_f32r(np.ascontiguousarray(input_factor.astype(np.float32)))
    in_maps = []
    for c in range(NCORES):
        sl = slice(c * OS, (c + 1) * OS)
        in_maps.append(
            {
                "w": np.ascontiguousarray(weight[sl].astype(np.float32)),
                "xT": xT,
                "wsT": round_f32r(
                    np.ascontiguousarray(
                        weight_scale[sl].T.astype(np.float32)
                    )
                ),
                "f": f,
            }
        )
    return in_maps


def gather_out(results):
    outs = [results[c]["out"] for c in range(NCORES)]
    full = np.concatenate(outs, axis=1)  # [T, OUT]
    return np.ascontiguousarray(full.reshape(B, S, OUT).astype(np.float32))


def kernel(x, weight, weight_scale, input_factor):
    global _NC_CACHE
    from concourse.bass_utils import run_bass_kernel_spmd

    if _NC_CACHE is None:
        _NC_CACHE = build_nc()
    nc = _NC_CACHE

    in_maps = make_in_maps(x, weight, weight_scale, input_factor)
    res = run_bass_kernel_spmd(nc, in_maps, core_ids=list(range(NCORES)))
    return gather_out(res.results)


if __name__ == "__main__":
    # quick self-run with random data
    rng = np.random.default_rng(0)
    x = rng.standard_normal((B, S, IN), dtype=np.float32)
    w = rng.standard_normal((OUT, IN), dtype=np.float32)
    ws = rng.standard_normal((OUT, R), dtype=np.float32)
    f = rng.standard_normal((R, IN), dtype=np.float32)
    out = kernel(x=x, weight=w, weight_scale=ws, input_factor=f)
    wv = ws @ f
    expected = np.einsum("bsi,oi->bso", x, np.sign(w) * wv)
    rel = np.abs(out - expected).max() / np.abs(expected).max()
    print("rel err:", rel)
